# revision 25
# baseline (speedup 1.0000x reference)
"""Trainium2 Bass kernel for nn_AttentionBlock (GroupNorm -> QKV -> cross+self
attention -> back projection + residual).

Sharding: data-parallel over batch B=8, one batch element per NeuronCore.

v2 design notes (vs baseline): the hard floor is the Scalar (ACT) engine --
8 heads x 16 s-chunks of exp over [128,1024] ~= 147us. Everything else is
arranged to hide under that stream:
  * x is loaded as bf16, DMAs spread over 4 engine queues by priority so
    the first exp can issue at ~9us.
  * GroupNorm stats run on gpsimd (sum x^2 via scalar_tensor_tensor accum)
    and DVE (sum x via tensor_scalar accum) per 128-channel chunk as x
    lands; rstd = exp(-0.5*ln(var+eps)) on ACT so the whole kernel uses a
    single activation table set (natural_log_exp).
  * per-head pipeline: scores (PE, K=64 row-tiles) -> exp (ACT) -> PV with
    augmented ones-column producing Z (PE), psum: sc ring 2x[128,1024]
    (4 banks) + pv [65,1024] (2 banks) + proj ping-pong (2 banks) = 8.
  * 1/Z via reciprocal_approx_fast + gpsimd partition_broadcast (no DRAM
    bounce); back projection + residual run at the end on all 8 banks.
"""

import contextlib
import functools

import numpy as np
import ml_dtypes

import concourse.bacc as bacc
import concourse.bass as bass
import concourse.tile as tile
from concourse import mybir
from concourse import bass_utils

BF16 = ml_dtypes.bfloat16
F32 = mybir.dt.float32
BF = mybir.dt.bfloat16
AF = mybir.ActivationFunctionType
ALU = mybir.AluOpType
AX = mybir.AxisListType

C = 512
T = 1024
S = 1024
NH = 8
HS = 64
EPS = 1e-5
NK = 4          # 128-partition channel chunks
NSC = 16        # 128-row score s-chunks (self 0..7, cond 8..15)
GSIZE = 16      # channels per group
CHUNK_ORDER = list(range(8, 16)) + list(range(8))   # cond chunks first


def _build_body(nc, tc, d, sbuf):
    pers = sbuf.enter_context(tc.tile_pool(name="pers", bufs=1))
    work = sbuf.enter_context(tc.tile_pool(name="work", bufs=2))
    epool = sbuf.enter_context(tc.tile_pool(name="epool", bufs=6))
    rzpool = sbuf.enter_context(tc.tile_pool(name="rzpool", bufs=2))
    outp = sbuf.enter_context(tc.tile_pool(name="outp", bufs=2))

    # ---------------- DMA loads (priority-ordered, 4 queues) ----------------
    # sync: x0,x1 + small tensors (+ output stores later)
    # vector: x2,x3           scalar: cond + late weights
    # gpsimd: early weights   tensor: kept clean for matmuls
    x_sb = []
    for j in range(NK):
        t_ = pers.tile([128, T], BF, tag=f"x{j}", name=f"x_sb{j}")
        x_sb.append(t_)
    nc.sync.dma_start(x_sb[0][:], d["x"][0:128, :])
    nc.sync.dma_start(x_sb[1][:], d["x"][128:256, :])
    nc.gpsimd.dma_start(x_sb[2][:], d["x"][256:384, :])
    nc.gpsimd.dma_start(x_sb[3][:], d["x"][384:512, :])

    cond_sb = []
    for j in range(NK):
        t_ = pers.tile([128, S], BF, tag=f"cond{j}", name=f"cond_sb{j}")
        nc.scalar.dma_start(t_[:], d["cond"][128 * j:128 * (j + 1), :])
        cond_sb.append(t_)

    def load_w(key, eng, kks=range(NK), tiles=None):
        tiles = tiles if tiles is not None else [None] * NK
        for kk in kks:
            t_ = pers.tile([128, 512], BF, tag=f"{key}{kk}", name=f"{key}_sb{kk}")
            eng.dma_start(t_[:], d[key][128 * kk:128 * (kk + 1), :])
            tiles[kk] = t_
        return tiles

    wkc_sb = load_w("wkc", nc.sync, kks=(0, 1))
    load_w("wkc", nc.gpsimd, kks=(2, 3), tiles=wkc_sb)
    wq_sb = load_w("wq", nc.gpsimd)
    wvc_sb = load_w("wvc", nc.scalar)
    wk_sb = load_w("wk", nc.gpsimd)
    wv_sb = load_w("wv", nc.sync)
    wb_sb = load_w("wb", nc.sync)

    def load_small(key, shape):
        t_ = pers.tile(shape, F32, tag=key, name=f"{key}_sb")
        nc.sync.dma_start(t_[:], d[key][:])
        return t_

    gamma_sb = load_small("gamma", [128, 4])
    beta_sb = load_small("beta", [128, 4])
    sel_f = load_small("sel_f", [128, 8])
    sel_b = load_small("sel_b", [8, 128])
    bq_sb = load_small("bq", [128, 4])
    bk_sb = load_small("bk", [128, 4])
    bkc_sb = load_small("bkc", [128, 4])
    bb_sb = load_small("bb", [128, 4])

    # v-biases broadcast across partitions (added to v during the vt drain)
    bvb = pers.tile([128, 512], BF, tag="bvb", name="bvb")
    src_ = d["bvh"][:]
    nc.sync.dma_start(bvb[:], bass.AP(tensor=src_.tensor, offset=src_.offset,
                                      ap=[[0, 128], [1, 512]]))
    bvcb = pers.tile([128, 512], BF, tag="bvcb", name="bvcb")
    src_ = d["bvch"][:]
    nc.sync.dma_start(bvcb[:], bass.AP(tensor=src_.tensor, offset=src_.offset,
                                       ap=[[0, 128], [1, 512]]))

    # ---------------- persistent SBUF tiles ----------------
    stats = pers.tile([128, 8], F32, tag="stats", name="stats")
    epsc = pers.tile([128, 1], F32, tag="epsc", name="epsc")
    nc.vector.memset(epsc[:], EPS)
    xn_sb = [pers.tile([128, T], BF, tag=f"xn{j}", name=f"xn_sb{j}")
             for j in range(NK)]
    kc_sb = [pers.tile([128, T], BF, tag=f"kc{m}", name=f"kc_sb{m}")
             for m in range(4)]
    q_sb = [pers.tile([128, T], BF, tag=f"q{m}", name=f"q_sb{m}")
            for m in range(4)]
    k_sb = [pers.tile([128, T], BF, tag=f"k{m}", name=f"k_sb{m}")
            for m in range(4)]
    vt_sb = [pers.tile([128, 8, 65], BF, tag=f"vt{i}", name=f"vt_sb{i}")
             for i in range(NSC)]
    attn_sb = [pers.tile([128, T], BF, tag=f"attn{p}", name=f"attn_sb{p}")
               for p in range(4)]

    # ---------------- psum pools (8 banks exactly) ----------------
    # sc ring 2x[128,1024] (4 banks) + pv 2x[65,1024] (4 banks); projections
    # and the tiny GN matmuls borrow sc-ring tiles.
    ps_stack = sbuf.enter_context(contextlib.ExitStack())
    ps_sc = ps_stack.enter_context(
        tc.tile_pool(name="ps_sc", bufs=2, space="PSUM"))
    ps_pv = ps_stack.enter_context(
        tc.tile_pool(name="ps_pv", bufs=2, space="PSUM"))

    # ---------------- GroupNorm stats (as x chunks land) ----------------
    for j in range(NK):
        sq = work.tile([128, T], BF, tag="sq", name=f"sq{j}")
        nc.vector.scalar_tensor_tensor(
            sq[:], x_sb[j][:], 1.0, x_sb[j][:], op0=ALU.mult, op1=ALU.mult,
            accum_out=stats[:, 4 + j:5 + j])
        xc = work.tile([128, T], BF, tag="xc", name=f"xc{j}")
        nc.vector.tensor_scalar(xc[:], x_sb[j][:], 1.0, 0.0, op0=ALU.mult,
                                op1=ALU.add, accum_out=stats[:, j:j + 1])
    for i in range(NSC):
        nc.gpsimd.memset(vt_sb[i][:, :, 64:65], 1.0)

    # ---------------- projections (borrow sc-ring psum tiles) ----------------
    pj_n = [0]

    def proj_m(w_tiles, rhs_tiles, bias_sb, out_sb, m):
        pj_n[0] += 1
        ps = ps_sc.tile([128, T], F32, tag="sc", name=f"pj{pj_n[0]}")
        for t2 in range(2):
            for kk in range(NK):
                nc.tensor.matmul(
                    ps[:, 512 * t2:512 * (t2 + 1)],
                    w_tiles[kk][:, 128 * m:128 * (m + 1)],
                    rhs_tiles[kk][:, 512 * t2:512 * (t2 + 1)],
                    start=(kk == 0), stop=(kk == NK - 1))
        nc.vector.tensor_scalar(out_sb[:], ps[:], bias_sb[:, m:m + 1], None,
                                op0=ALU.add)

    proj_m(wkc_sb, cond_sb, bkc_sb, kc_sb[0], 0)

    # preload the ln/exp activation table while DMAs stream
    preld = pers.tile([1, 1], F32, tag="preld", name="preld")
    nc.scalar.activation(preld[:], epsc[0:1, :], AF.Ln)

    # ---------------- GroupNorm combine + xn ----------------
    gpst = ps_sc.tile([128, T], F32, tag="sc", name="gpst")
    nc.tensor.matmul(gpst[0:8, 0:8], sel_f[:], stats[:], start=True, stop=True)
    gstats = pers.tile([8, 8], F32, tag="gstats", name="gstats")
    inv_n = 1.0 / (GSIZE * T)
    nc.vector.tensor_scalar_mul(gstats[:, 0:8], gpst[0:8, 0:8], inv_n)
    var = pers.tile([8, 4], F32, tag="var", name="var")
    nc.vector.tensor_mul(var[:], gstats[:, 0:4], gstats[:, 0:4])
    nc.vector.tensor_sub(var[:], gstats[:, 4:8], var[:])
    # rstd = exp(-0.5 * ln(var + eps)): stays in the exp/ln table set
    nc.scalar.activation(var[:], var[:], AF.Ln, bias=epsc[0:8, :])
    nc.scalar.activation(gstats[:, 4:8], var[:], AF.Exp, scale=-0.5)
    bpst = ps_sc.tile([128, T], F32, tag="sc", name="bpst")
    nc.tensor.matmul(bpst[0:128, 0:8], sel_b[:], gstats[:], start=True,
                     stop=True)
    scale = pers.tile([128, 4], F32, tag="scale", name="scale")
    shift = pers.tile([128, 4], F32, tag="shift", name="shift")
    nc.vector.tensor_mul(scale[:], gamma_sb[:], bpst[0:128, 4:8])
    nc.vector.tensor_mul(shift[:], bpst[0:128, 0:4], scale[:])
    nc.vector.tensor_sub(shift[:], beta_sb[:], shift[:])
    for j in range(NK):
        nc.vector.tensor_scalar(xn_sb[j][:], x_sb[j][:], scale[:, j:j + 1],
                                shift[:, j:j + 1], op0=ALU.mult, op1=ALU.add)

    proj_m(wq_sb, xn_sb, bq_sb, q_sb[0], 0)

    # ---------------- attention machinery ----------------
    def emit_vt(i):
        m8 = i % 8
        src = xn_sb if i < 8 else cond_sb
        w = wv_sb if i < 8 else wvc_sb
        bcast = bvb if i < 8 else bvcb
        pst = ps_sc.tile([128, T], F32, tag="sc", name=f"pj_vt{i}")
        ps = pst[:, 0:512]
        for kk in range(NK):
            nc.tensor.matmul(ps, src[kk][:, 128 * m8:128 * (m8 + 1)],
                             w[kk][:], start=(kk == 0), stop=(kk == NK - 1))
        nc.vector.tensor_add(
            vt_sb[i][:, :, 0:64],
            ps.rearrange("p (h c) -> p h c", h=NH),
            bcast[:].rearrange("p (h c) -> p h c", h=NH))

    def emit_chunk(h, i, pvt, ci):
        """scores -> exp -> PV for head h, s-chunk i."""
        rb = 64 * (h % 2)
        ksrc = k_sb[h // 2] if i < 8 else kc_sb[h // 2]
        scol = 128 * (i % 8)
        sc = ps_sc.tile([128, T], F32, tag="sc", name=f"sc{h}_{i}")
        for t2 in range(2):
            nc.tensor.matmul(
                sc[:, 512 * t2:512 * (t2 + 1)],
                ksrc[rb:rb + 64, scol:scol + 128],
                q_sb[h // 2][rb:rb + 64, 512 * t2:512 * (t2 + 1)],
                start=True, stop=True)
        e = epool.tile([128, T], BF, tag="e", name=f"e{h}_{i}")
        nc.scalar.activation(e[:], sc[:], AF.Exp, scale=0.125)
        for t2 in range(2):
            nc.tensor.matmul(
                pvt[:, 512 * t2:512 * (t2 + 1)],
                vt_sb[i][:, h, :],
                e[:, 512 * t2:512 * (t2 + 1)],
                start=(ci == 0), stop=(ci == NSC - 1))

    def head_drain(h, pvt):
        """pv psum row 64 = Z; broadcast 1/Z over 64 partitions and fold the
        normalize into the psum->SBUF drain."""
        p, hp = h // 2, h % 2
        zrow = rzpool.tile([1, T], F32, tag="zrow", name=f"zrow{h}")
        nc.vector.tensor_copy(zrow[:], pvt[64:65, 0:T])
        zb = rzpool.tile([64, T], F32, tag="zb", name=f"zb{h}")
        nc.gpsimd.partition_broadcast(zb[:], zrow[0:1, :])
        rz = rzpool.tile([64, T], F32, tag="rz", name=f"rz{h}")
        nc.vector.reciprocal_approx_fast(rz[:], zb[:])
        nc.vector.tensor_mul(attn_sb[p][64 * hp:64 * (hp + 1), :],
                             pvt[0:64, 0:T], rz[:])

    # ---------------- head loop ----------------
    for h in range(NH):
        pvt = ps_pv.tile([65, T], F32, tag="pv", name=f"pv{h}")
        for ci, i in enumerate(CHUNK_ORDER):
            if h == 0 and i >= 8:
                emit_vt(i)                      # cond-side v while ACT warms
            if h == 0 and i == 0:
                proj_m(wk_sb, xn_sb, bk_sb, k_sb[0], 0)
            if h == 0 and i < 8:
                emit_vt(i)
            emit_chunk(h, i, pvt, ci)
        head_drain(h, pvt)
        # deferred projections for upcoming heads, hidden under the ACT stream
        if h in (1, 3, 5):
            m = (h + 1) // 2
            proj_m(wkc_sb, cond_sb, bkc_sb, kc_sb[m], m)
            proj_m(wq_sb, xn_sb, bq_sb, q_sb[m], m)
            proj_m(wk_sb, xn_sb, bk_sb, k_sb[m], m)

    # ---------------- back projection + residual ----------------
    ps_stack.close()
    with tc.tile_pool(name="ps_bk", bufs=1, space="PSUM") as ps_bk:
        for m in range(4):
            outsb = outp.tile([128, T], F32, tag="outsb", name=f"outsb{m}")
            for t2 in range(2):
                bk = ps_bk.tile([128, 512], F32, tag=f"bk{m}{t2}",
                                name=f"ps_bk{m}{t2}")
                for kk in range(NK):
                    nc.tensor.matmul(bk[:],
                                     wb_sb[kk][:, 128 * m:128 * (m + 1)],
                                     attn_sb[kk][:, 512 * t2:512 * (t2 + 1)],
                                     start=(kk == 0), stop=(kk == NK - 1))
                nc.vector.scalar_tensor_tensor(
                    outsb[:, 512 * t2:512 * (t2 + 1)], bk[:],
                    bb_sb[:, m:m + 1],
                    x_sb[m][:, 512 * t2:512 * (t2 + 1)],
                    op0=ALU.add, op1=ALU.add)
                eng = nc.sync if t2 == 0 else nc.gpsimd
                eng.dma_start(
                    d["out"][128 * m:128 * (m + 1), 512 * t2:512 * (t2 + 1)],
                    outsb[:, 512 * t2:512 * (t2 + 1)])


@functools.lru_cache(maxsize=1)
def _build():
    nc = bacc.Bacc("TRN2", target_bir_lowering=False, debug=False)
    d = {}
    d["x"] = nc.dram_tensor("x", [C, T], BF, kind="ExternalInput")
    d["cond"] = nc.dram_tensor("cond", [512, S], BF, kind="ExternalInput")
    for w in ("wq", "wk", "wkc", "wv", "wvc", "wb"):
        d[w] = nc.dram_tensor(w, [512, 512], BF, kind="ExternalInput")
    for v in ("gamma", "beta", "bq", "bk", "bkc", "bb"):
        d[v] = nc.dram_tensor(v, [128, 4], F32, kind="ExternalInput")
    d["bvh"] = nc.dram_tensor("bvh", [1, 512], BF, kind="ExternalInput")
    d["bvch"] = nc.dram_tensor("bvch", [1, 512], BF, kind="ExternalInput")
    d["sel_f"] = nc.dram_tensor("sel_f", [128, 8], F32, kind="ExternalInput")
    d["sel_b"] = nc.dram_tensor("sel_b", [8, 128], F32, kind="ExternalInput")
    d["out"] = nc.dram_tensor("out", [C, T], F32, kind="ExternalOutput")

    with tile.TileContext(nc) as tc:
        with contextlib.ExitStack() as sbuf:
            _build_body(nc, tc, d, sbuf)
    nc.compile()
    return nc


def _prep_shared(gn_gamma, gn_beta, Wf, bf, Wt, bt, Wb, bb):
    f32 = np.float32
    Wf_r = np.asarray(Wf, f32).reshape(8, 3, 64, 512)
    Wt_r = np.asarray(Wt, f32).reshape(8, 2, 64, 512)
    bf_r = np.asarray(bf, f32).reshape(8, 3, 64)
    bt_r = np.asarray(bt, f32).reshape(8, 2, 64)

    def wT(a):  # [512(out), 512(in)] -> [in, out] bf16
        return np.ascontiguousarray(a.reshape(512, 512).T).astype(BF16)

    def pcol(v):  # [512] -> [128, 4]
        return np.ascontiguousarray(np.asarray(v, f32).reshape(4, 128).T)

    sel_f = (np.arange(128)[:, None] // GSIZE ==
             np.arange(8)[None, :]).astype(f32)
    return {
        "wq": wT(Wf_r[:, 0]),
        "wk": wT(Wf_r[:, 1]),
        "wv": wT(Wf_r[:, 2]),
        "wkc": wT(Wt_r[:, 0]),
        "wvc": wT(Wt_r[:, 1]),
        "wb": np.ascontiguousarray(np.asarray(Wb, f32).T).astype(BF16),
        "gamma": pcol(gn_gamma),
        "beta": pcol(gn_beta),
        "bq": pcol(bf_r[:, 0].reshape(512)),
        "bk": pcol(bf_r[:, 1].reshape(512)),
        "bkc": pcol(bt_r[:, 0].reshape(512)),
        "bb": pcol(bb),
        "bvh": np.ascontiguousarray(bf_r[:, 2].reshape(1, 512)).astype(BF16),
        "bvch": np.ascontiguousarray(bt_r[:, 1].reshape(1, 512)).astype(BF16),
        "sel_f": sel_f,
        "sel_b": np.ascontiguousarray(sel_f.T),
    }


def _run(inputs, trace=False, tmpdir=None):
    nc = _build()
    shared = _prep_shared(inputs["gn_gamma"], inputs["gn_beta"],
                          inputs["Wf"], inputs["bf"], inputs["Wt"],
                          inputs["bt"], inputs["Wb"], inputs["bb"])
    feat = np.asarray(inputs["input_feature"], np.float32)
    cond = np.asarray(inputs["attention_condition"], np.float32)
    in_maps = []
    for b in range(8):
        m = dict(shared)
        m["x"] = feat[b].reshape(C, T).astype(BF16)
        m["cond"] = cond[b].astype(BF16)
        in_maps.append(m)
    res = bass_utils.run_bass_kernel_spmd(nc, in_maps, core_ids=list(range(8)),
                                          trace=trace, tmpdir=tmpdir)
    out = np.stack([r["out"] for r in res.results], axis=0)
    return out.reshape(8, C, 32, 32).astype(np.float32), res


def kernel(**inputs):
    out, _ = _run(inputs, trace=False)
    return out


# revision 35
# speedup vs baseline: 1.2926x; 1.2926x over previous
"""Trainium2 Bass kernel for nn_AttentionBlock (GroupNorm -> QKV -> cross+self
attention -> back projection + residual).

Sharding: data-parallel over batch B=8, one batch element per NeuronCore.

v2 design notes (vs baseline): the hard floor is the Scalar (ACT) engine --
8 heads x 16 s-chunks of exp over [128,1024] ~= 147us. Everything else is
arranged to hide under that stream:
  * x is loaded as bf16, DMAs spread over 4 engine queues by priority so
    the first exp can issue at ~9us.
  * GroupNorm stats run on gpsimd (sum x^2 via scalar_tensor_tensor accum)
    and DVE (sum x via tensor_scalar accum) per 128-channel chunk as x
    lands; rstd = exp(-0.5*ln(var+eps)) on ACT so the whole kernel uses a
    single activation table set (natural_log_exp).
  * per-head pipeline: scores (PE, K=64 row-tiles) -> exp (ACT) -> PV with
    augmented ones-column producing Z (PE), psum: sc ring 2x[128,1024]
    (4 banks) + pv [65,1024] (2 banks) + proj ping-pong (2 banks) = 8.
  * 1/Z via reciprocal_approx_fast + gpsimd partition_broadcast (no DRAM
    bounce); back projection + residual run at the end on all 8 banks.
"""

import contextlib
import functools

import numpy as np
import ml_dtypes

import concourse.bacc as bacc
import concourse.bass as bass
import concourse.tile as tile
from concourse import mybir
from concourse import bass_utils

BF16 = ml_dtypes.bfloat16
F32 = mybir.dt.float32
BF = mybir.dt.bfloat16
AF = mybir.ActivationFunctionType
ALU = mybir.AluOpType
AX = mybir.AxisListType

C = 512
T = 1024
S = 1024
NH = 8
HS = 64
EPS = 1e-5
NK = 4          # 128-partition channel chunks
NSC = 16        # 128-row score s-chunks (self 0..7, cond 8..15)
GSIZE = 16      # channels per group
CHUNK_ORDER = list(range(8, 16)) + list(range(8))   # cond chunks first


def _build_body(nc, tc, d, sbuf):
    pers = sbuf.enter_context(tc.tile_pool(name="pers", bufs=1))
    work = sbuf.enter_context(tc.tile_pool(name="work", bufs=2))
    epool = sbuf.enter_context(tc.tile_pool(name="epool", bufs=6))
    rzpool = sbuf.enter_context(tc.tile_pool(name="rzpool", bufs=2))
    outp = sbuf.enter_context(tc.tile_pool(name="outp", bufs=2))

    # ---------------- DMA loads (priority-ordered, 4 queues) ----------------
    # sync: x0,x1 + small tensors (+ output stores later)
    # vector: x2,x3           scalar: cond + late weights
    # gpsimd: early weights   tensor: kept clean for matmuls
    x_sb = []
    for j in range(NK):
        t_ = pers.tile([128, T], BF, tag=f"x{j}", name=f"x_sb{j}")
        x_sb.append(t_)
    nc.sync.dma_start(x_sb[0][:], d["x"][0:128, :])
    nc.sync.dma_start(x_sb[1][:], d["x"][128:256, :])
    nc.gpsimd.dma_start(x_sb[2][:], d["x"][256:384, :])
    nc.gpsimd.dma_start(x_sb[3][:], d["x"][384:512, :])

    cond_sb = []
    for j in range(NK):
        t_ = pers.tile([128, S], BF, tag=f"cond{j}", name=f"cond_sb{j}")
        nc.scalar.dma_start(t_[:], d["cond"][128 * j:128 * (j + 1), :])
        cond_sb.append(t_)

    def load_w(key, eng, kks=range(NK), tiles=None):
        tiles = tiles if tiles is not None else [None] * NK
        for kk in kks:
            t_ = pers.tile([128, 512], BF, tag=f"{key}{kk}", name=f"{key}_sb{kk}")
            eng.dma_start(t_[:], d[key][128 * kk:128 * (kk + 1), :])
            tiles[kk] = t_
        return tiles

    wkc_sb = load_w("wkc", nc.sync, kks=(0, 1))
    load_w("wkc", nc.gpsimd, kks=(2, 3), tiles=wkc_sb)
    wq_sb = load_w("wq", nc.gpsimd)
    wk_sb = load_w("wk", nc.gpsimd)

    def load_small(key, shape):
        t_ = pers.tile(shape, F32, tag=key, name=f"{key}_sb")
        nc.sync.dma_start(t_[:], d[key][:])
        return t_

    gamma_sb = load_small("gamma", [128, 4])
    beta_sb = load_small("beta", [128, 4])
    sel_f = load_small("sel_f", [128, 8])
    sel_b = load_small("sel_b", [8, 128])
    bq_sb = load_small("bq", [128, 4])
    bk_sb = load_small("bk", [128, 4])
    bkc_sb = load_small("bkc", [128, 4])
    bb_sb = load_small("bb", [128, 4])

    # v-biases broadcast across partitions (added to v during the vt drain)
    bvb = pers.tile([128, 512], BF, tag="bvb", name="bvb")
    src_ = d["bvh"][:]
    nc.sync.dma_start(bvb[:], bass.AP(tensor=src_.tensor, offset=src_.offset,
                                      ap=[[0, 128], [1, 512]]))
    bvcb = pers.tile([128, 512], BF, tag="bvcb", name="bvcb")
    src_ = d["bvch"][:]
    nc.sync.dma_start(bvcb[:], bass.AP(tensor=src_.tensor, offset=src_.offset,
                                       ap=[[0, 128], [1, 512]]))

    wvc_sb = load_w("wvc", nc.sync)
    wv_sb = load_w("wv", nc.scalar)
    wb_sb = load_w("wb", nc.scalar)

    # ---------------- persistent SBUF tiles ----------------
    stats = pers.tile([128, 8], F32, tag="stats", name="stats")
    epsc = pers.tile([128, 1], F32, tag="epsc", name="epsc")
    nc.vector.memset(epsc[:], EPS)
    xn_sb = [pers.tile([128, T], BF, tag=f"xn{j}", name=f"xn_sb{j}")
             for j in range(NK)]
    kc_sb = [pers.tile([128, T], BF, tag=f"kc{m}", name=f"kc_sb{m}")
             for m in range(4)]
    q_sb = [pers.tile([128, T], BF, tag=f"q{m}", name=f"q_sb{m}")
            for m in range(4)]
    k_sb = [pers.tile([128, T], BF, tag=f"k{m}", name=f"k_sb{m}")
            for m in range(4)]
    vt_sb = [pers.tile([128, 8, 65], BF, tag=f"vt{i}", name=f"vt_sb{i}")
             for i in range(NSC)]
    attn_sb = [pers.tile([128, T], BF, tag=f"attn{p}", name=f"attn_sb{p}")
               for p in range(4)]

    # ---------------- psum pools (8 banks exactly) ----------------
    # sc ring 2x[128,1024] (4 banks) + pv [65,1024] (2 banks) + proj/vt
    # ping-pong (2 banks). PV matmuls trail the score/exp stream by TRAIL
    # chunks so the pv drain chain never blocks the PE queue ahead of the
    # next head's scores.
    ps_stack = sbuf.enter_context(contextlib.ExitStack())
    ps_sc = ps_stack.enter_context(
        tc.tile_pool(name="ps_sc", bufs=2, space="PSUM"))
    ps_pv = ps_stack.enter_context(
        tc.tile_pool(name="ps_pv", bufs=1, space="PSUM"))
    ps_pj = ps_stack.enter_context(
        tc.tile_pool(name="ps_pj", bufs=2, space="PSUM"))

    # ---------------- GroupNorm stats (as x chunks land) ----------------
    for j in range(NK):
        sq = work.tile([128, T], BF, tag="sq", name=f"sq{j}")
        nc.vector.scalar_tensor_tensor(
            sq[:], x_sb[j][:], 1.0, x_sb[j][:], op0=ALU.mult, op1=ALU.mult,
            accum_out=stats[:, 4 + j:5 + j])
        xc = work.tile([128, T], BF, tag="xc", name=f"xc{j}")
        nc.vector.tensor_scalar(xc[:], x_sb[j][:], 1.0, 0.0, op0=ALU.mult,
                                op1=ALU.add, accum_out=stats[:, j:j + 1])
    for i in range(NSC):
        nc.gpsimd.memset(vt_sb[i][:, :, 64:65], 1.0)

    # ---------------- projections (borrow sc-ring psum tiles) ----------------
    pj_n = [0]

    def proj_m(w_tiles, rhs_tiles, bias_sb, out_sb, m):
        for t2 in range(2):
            pj_n[0] += 1
            ps = ps_pj.tile([128, 512], F32, tag="proj", name=f"pj{pj_n[0]}")
            for kk in range(NK):
                nc.tensor.matmul(
                    ps[:], w_tiles[kk][:, 128 * m:128 * (m + 1)],
                    rhs_tiles[kk][:, 512 * t2:512 * (t2 + 1)],
                    start=(kk == 0), stop=(kk == NK - 1))
            nc.vector.tensor_scalar(
                out_sb[:, 512 * t2:512 * (t2 + 1)], ps[:],
                bias_sb[:, m:m + 1], None, op0=ALU.add)

    proj_m(wkc_sb, cond_sb, bkc_sb, kc_sb[0], 0)

    # preload the ln/exp activation table while DMAs stream
    preld = pers.tile([1, 1], F32, tag="preld", name="preld")
    nc.scalar.activation(preld[:], epsc[0:1, :], AF.Ln)

    # ---------------- attention machinery ----------------
    def emit_vt(i):
        m8 = i % 8
        src = xn_sb if i < 8 else cond_sb
        w = wv_sb if i < 8 else wvc_sb
        bcast = bvb if i < 8 else bvcb
        ps = ps_pj.tile([128, 512], F32, tag="proj", name=f"pj_vt{i}")
        for kk in range(NK):
            nc.tensor.matmul(ps[:], src[kk][:, 128 * m8:128 * (m8 + 1)],
                             w[kk][:], start=(kk == 0), stop=(kk == NK - 1))
        nc.vector.tensor_add(
            vt_sb[i][:, :, 0:64],
            ps[:].rearrange("p (h c) -> p h c", h=NH),
            bcast[:].rearrange("p (h c) -> p h c", h=NH))

    def emit_scores(h, i):
        """scores -> exp for head h, s-chunk i; returns the e tile."""
        rb = 64 * (h % 2)
        ksrc = k_sb[h // 2] if i < 8 else kc_sb[h // 2]
        scol = 128 * (i % 8)
        sc = ps_sc.tile([128, T], F32, tag="sc", name=f"sc{h}_{i}")
        for t2 in range(2):
            nc.tensor.matmul(
                sc[:, 512 * t2:512 * (t2 + 1)],
                ksrc[rb:rb + 64, scol:scol + 128],
                q_sb[h // 2][rb:rb + 64, 512 * t2:512 * (t2 + 1)],
                start=True, stop=True)
        e = epool.tile([128, T], BF, tag="e", name=f"e{h}_{i}")
        nc.scalar.activation(e[:], sc[:], AF.Exp, scale=0.125)
        return e

    def emit_pv(h, pvt, e, i, ci):
        for t2 in range(2):
            nc.tensor.matmul(
                pvt[:, 512 * t2:512 * (t2 + 1)],
                vt_sb[i][:, h, :],
                e[:, 512 * t2:512 * (t2 + 1)],
                start=(ci == 0), stop=(ci == NSC - 1))

    def head_drain(h, pvt):
        """pv psum row 64 = Z; broadcast 1/Z over 64 partitions and fold the
        normalize into the psum->SBUF drain."""
        p, hp = h // 2, h % 2
        zrow = rzpool.tile([1, T], F32, tag="zrow", name=f"zrow{h}")
        nc.vector.tensor_copy(zrow[:], pvt[64:65, 0:T])
        zb = rzpool.tile([64, T], F32, tag="zb", name=f"zb{h}")
        nc.gpsimd.partition_broadcast(zb[:], zrow[0:1, :])
        rz = rzpool.tile([64, T], F32, tag="rz", name=f"rz{h}")
        nc.vector.reciprocal_approx_fast(rz[:], zb[:])
        nc.vector.tensor_mul(attn_sb[p][64 * hp:64 * (hp + 1), :],
                             pvt[0:64, 0:T], rz[:])

    # ---------------- GroupNorm combine + xn ----------------
    gpst = ps_pj.tile([128, 512], F32, tag="proj", name="gpst")
    nc.tensor.matmul(gpst[0:8, 0:8], sel_f[:], stats[:], start=True, stop=True)
    gstats = pers.tile([8, 8], F32, tag="gstats", name="gstats")
    inv_n = 1.0 / (GSIZE * T)
    nc.vector.tensor_scalar_mul(gstats[:, 0:8], gpst[0:8, 0:8], inv_n)
    var = pers.tile([8, 4], F32, tag="var", name="var")
    nc.vector.tensor_mul(var[:], gstats[:, 0:4], gstats[:, 0:4])
    nc.vector.tensor_sub(var[:], gstats[:, 4:8], var[:])
    # rstd = exp(-0.5 * ln(var + eps)): stays in the exp/ln table set
    nc.scalar.activation(var[:], var[:], AF.Ln, bias=epsc[0:8, :])
    nc.scalar.activation(gstats[:, 4:8], var[:], AF.Exp, scale=-0.5)
    bpst = ps_pj.tile([128, 512], F32, tag="proj", name="bpst")
    nc.tensor.matmul(bpst[0:128, 0:8], sel_b[:], gstats[:], start=True,
                     stop=True)
    scale = pers.tile([128, 4], F32, tag="scale", name="scale")
    shift = pers.tile([128, 4], F32, tag="shift", name="shift")
    nc.vector.tensor_mul(scale[:], gamma_sb[:], bpst[0:128, 4:8])
    nc.vector.tensor_mul(shift[:], bpst[0:128, 0:4], scale[:])
    nc.vector.tensor_sub(shift[:], beta_sb[:], shift[:])
    for j in range(NK):
        nc.vector.tensor_scalar(xn_sb[j][:], x_sb[j][:], scale[:, j:j + 1],
                                shift[:, j:j + 1], op0=ALU.mult, op1=ALU.add)

    for i in range(8, NSC):
        emit_vt(i)          # cond-side v: fills the PE while xn resolves
    proj_m(wq_sb, xn_sb, bq_sb, q_sb[0], 0)

    # ---------------- head loop ----------------
    TRAIL = 4   # PV trails scores/exp by this many chunks
    for h in range(NH):
        pvt = ps_pv.tile([65, T], F32, tag="pv", name=f"pv{h}")
        etiles = []
        for ci, i in enumerate(CHUNK_ORDER):
            if h == 0 and i == 0:
                proj_m(wk_sb, xn_sb, bk_sb, k_sb[0], 0)
            if h == 0 and i < 8:
                emit_vt(i)                  # one self-side vt per chunk slot
            etiles.append(emit_scores(h, i))
            if ci >= TRAIL:
                cj = ci - TRAIL
                emit_pv(h, pvt, etiles[cj], CHUNK_ORDER[cj], cj)
        for cj in range(NSC - TRAIL, NSC):
            emit_pv(h, pvt, etiles[cj], CHUNK_ORDER[cj], cj)
        head_drain(h, pvt)
        # deferred projections for upcoming heads, hidden under the ACT stream
        if h < 6:
            m = h // 2 + 1
            if h % 2 == 0:
                proj_m(wkc_sb, cond_sb, bkc_sb, kc_sb[m], m)
            else:
                proj_m(wq_sb, xn_sb, bq_sb, q_sb[m], m)
                proj_m(wk_sb, xn_sb, bk_sb, k_sb[m], m)

    # ---------------- back projection + residual ----------------
    ps_stack.close()
    with tc.tile_pool(name="ps_bk", bufs=1, space="PSUM") as ps_bk:
        for m in range(4):
            outsb = outp.tile([128, T], F32, tag="outsb", name=f"outsb{m}")
            for t2 in range(2):
                bk = ps_bk.tile([128, 512], F32, tag=f"bk{m}{t2}",
                                name=f"ps_bk{m}{t2}")
                for kk in range(NK):
                    nc.tensor.matmul(bk[:],
                                     wb_sb[kk][:, 128 * m:128 * (m + 1)],
                                     attn_sb[kk][:, 512 * t2:512 * (t2 + 1)],
                                     start=(kk == 0), stop=(kk == NK - 1))
                nc.vector.scalar_tensor_tensor(
                    outsb[:, 512 * t2:512 * (t2 + 1)], bk[:],
                    bb_sb[:, m:m + 1],
                    x_sb[m][:, 512 * t2:512 * (t2 + 1)],
                    op0=ALU.add, op1=ALU.add)
                eng = nc.sync if t2 == 0 else nc.gpsimd
                eng.dma_start(
                    d["out"][128 * m:128 * (m + 1), 512 * t2:512 * (t2 + 1)],
                    outsb[:, 512 * t2:512 * (t2 + 1)])


@functools.lru_cache(maxsize=1)
def _build():
    nc = bacc.Bacc("TRN2", target_bir_lowering=False, debug=False)
    d = {}
    d["x"] = nc.dram_tensor("x", [C, T], BF, kind="ExternalInput")
    d["cond"] = nc.dram_tensor("cond", [512, S], BF, kind="ExternalInput")
    for w in ("wq", "wk", "wkc", "wv", "wvc", "wb"):
        d[w] = nc.dram_tensor(w, [512, 512], BF, kind="ExternalInput")
    for v in ("gamma", "beta", "bq", "bk", "bkc", "bb"):
        d[v] = nc.dram_tensor(v, [128, 4], F32, kind="ExternalInput")
    d["bvh"] = nc.dram_tensor("bvh", [1, 512], BF, kind="ExternalInput")
    d["bvch"] = nc.dram_tensor("bvch", [1, 512], BF, kind="ExternalInput")
    d["sel_f"] = nc.dram_tensor("sel_f", [128, 8], F32, kind="ExternalInput")
    d["sel_b"] = nc.dram_tensor("sel_b", [8, 128], F32, kind="ExternalInput")
    d["out"] = nc.dram_tensor("out", [C, T], F32, kind="ExternalOutput")

    with tile.TileContext(nc) as tc:
        with contextlib.ExitStack() as sbuf:
            _build_body(nc, tc, d, sbuf)
    nc.compile()
    return nc


def _prep_shared(gn_gamma, gn_beta, Wf, bf, Wt, bt, Wb, bb):
    f32 = np.float32
    Wf_r = np.asarray(Wf, f32).reshape(8, 3, 64, 512)
    Wt_r = np.asarray(Wt, f32).reshape(8, 2, 64, 512)
    bf_r = np.asarray(bf, f32).reshape(8, 3, 64)
    bt_r = np.asarray(bt, f32).reshape(8, 2, 64)

    def wT(a):  # [512(out), 512(in)] -> [in, out] bf16
        return np.ascontiguousarray(a.reshape(512, 512).T).astype(BF16)

    def pcol(v):  # [512] -> [128, 4]
        return np.ascontiguousarray(np.asarray(v, f32).reshape(4, 128).T)

    sel_f = (np.arange(128)[:, None] // GSIZE ==
             np.arange(8)[None, :]).astype(f32)
    return {
        "wq": wT(Wf_r[:, 0]),
        "wk": wT(Wf_r[:, 1]),
        "wv": wT(Wf_r[:, 2]),
        "wkc": wT(Wt_r[:, 0]),
        "wvc": wT(Wt_r[:, 1]),
        "wb": np.ascontiguousarray(np.asarray(Wb, f32).T).astype(BF16),
        "gamma": pcol(gn_gamma),
        "beta": pcol(gn_beta),
        "bq": pcol(bf_r[:, 0].reshape(512)),
        "bk": pcol(bf_r[:, 1].reshape(512)),
        "bkc": pcol(bt_r[:, 0].reshape(512)),
        "bb": pcol(bb),
        "bvh": np.ascontiguousarray(bf_r[:, 2].reshape(1, 512)).astype(BF16),
        "bvch": np.ascontiguousarray(bt_r[:, 1].reshape(1, 512)).astype(BF16),
        "sel_f": sel_f,
        "sel_b": np.ascontiguousarray(sel_f.T),
    }


def _run(inputs, trace=False, tmpdir=None):
    nc = _build()
    shared = _prep_shared(inputs["gn_gamma"], inputs["gn_beta"],
                          inputs["Wf"], inputs["bf"], inputs["Wt"],
                          inputs["bt"], inputs["Wb"], inputs["bb"])
    feat = np.asarray(inputs["input_feature"], np.float32)
    cond = np.asarray(inputs["attention_condition"], np.float32)
    in_maps = []
    for b in range(8):
        m = dict(shared)
        m["x"] = feat[b].reshape(C, T).astype(BF16)
        m["cond"] = cond[b].astype(BF16)
        in_maps.append(m)
    res = bass_utils.run_bass_kernel_spmd(nc, in_maps, core_ids=list(range(8)),
                                          trace=trace, tmpdir=tmpdir)
    out = np.stack([r["out"] for r in res.results], axis=0)
    return out.reshape(8, C, 32, 32).astype(np.float32), res


def kernel(**inputs):
    out, _ = _run(inputs, trace=False)
    return out
